# revision 10
# baseline (speedup 1.0000x reference)
"""Trainium2 kernel for the nn_Circuit coupled-mode ODE problem.

Math: dA/dt = i*diag(omega + gamma*|A|^2) A + T2 A, integrated t in [0,2],
sampled at 200 points; A is (1024 batch, 64 modes) complex, padded with ones
for modes 48..63.  L = T2 + i*diag(omega) is constant, nearly skew-Hermitian.

Device algorithm: Strang splitting, linear part exact via host-precomputed
matrix exponentials, nonlinear part as a per-element phase rotation with
sin(theta) ~= theta (theta = gamma*h*|A|^2 <= ~1e-2, cubic error ~1e-7).
|A|^2 must come from the CURRENT state each step -- every lag/extrapolation/
predictor variant fails numerically (|A|^2 beats at ~2h period via the stiff
~288i eigenvalue; host-validated).  The per-step serial path is therefore

  z -> s2=Square(z) [ACT] -> ss=Qg@s2 [PE pair-sum matmul, gh*sgn folded
  into the weights] -> qt=z*ss [V] -> chain matmuls [PE]

with everything else off-path: pp = z*cc uses a 2-step-lagged cosine
cc = 1 - theta^2/2 (validated, the cos factor deviates from 1 by O(1e-4)),
computed via ACT Square of ss and a V affine; y output and its PSUM->SBUF
copy overlap the path.

    z_{k+1} = Eh_hi@pp + Eh_lo@pp + EhP@qt     (fp16, f32 PSUM accum)
    y_{k+1} = Eh2@pp + Eh2P@qt                 -> ACT copy -> DMA out

Chain weights Eh are hi/lo fp16 compensated; other weights single fp16
(operands O(theta) or output-only).  Host-validated rel err ~2.6e-3.

State layout: (128 partitions, 128 batch) f32 PSUM, partition p = 2j+c
interleaving re/im of mode j.  Output written mode-major per core and
transposed on the host during unsharding.

Sharding: pure data parallel, batch 1024 = 8 cores x 128.
"""

import numpy as np

MODES = 64
INPUT_MODES = 48
BATCH = 1024
EVAL_PTS = 200
EPS = 1e-8
N_CORES = 8
B_LOC = BATCH // N_CORES  # 128
NT = EVAL_PTS - 1  # 199 intervals
DT = 2.0 / NT
GAMMA = 0.2
GH = GAMMA * DT

_CACHE = {}

# ---------------------------------------------------------------------------
# host-side math
# ---------------------------------------------------------------------------

def _t2_like_reference(params, omega, kappa):
    """Reproduce the reference's float32 jax computation of T2 exactly."""
    import jax

    try:
        cpu = jax.devices("cpu")[0]
    except Exception:
        cpu = None

    import contextlib

    ctx = jax.default_device(cpu) if cpu is not None else contextlib.nullcontext()
    with ctx:
        import jax.numpy as jnp

        n = MODES
        p = jnp.asarray(params, dtype=jnp.float32)
        n_off = n * (n - 1) // 2
        iu = jnp.triu_indices(n, 1)
        off = p[:n_off] + 1j * p[n_off:2 * n_off]
        H = jnp.zeros((n, n), dtype=jnp.complex64).at[iu].set(off.astype(jnp.complex64))
        H = H + H.conj().T
        d = p[2 * n_off:]
        diag = jnp.concatenate([d, -jnp.sum(d, keepdims=True)])
        H = H + jnp.diag(diag.astype(jnp.complex64))
        U = jax.scipy.linalg.expm(1j * H)
        I = jnp.eye(n, dtype=jnp.complex64)
        M = U.T @ U
        mix = M @ jnp.linalg.inv(I - M + EPS * I)
        T2 = -jnp.asarray(kappa, dtype=jnp.float32) * (
            0.5 * jnp.eye(n, dtype=jnp.float32) + mix
        )
        T2_re = np.asarray(jnp.real(T2), dtype=np.float32)
        T2_im = np.asarray(jnp.imag(T2), dtype=np.float32)
    return T2_re, T2_im


def _expm(M):
    w, V = np.linalg.eig(M)
    return (V * np.exp(w)) @ np.linalg.inv(V)


def _big_il(C):
    """Complex (64,64) -> real (128,128) operator in the interleaved re/im basis."""
    A = np.zeros((2 * MODES, 2 * MODES), dtype=np.float64)
    A[0::2, 0::2] = C.real
    A[0::2, 1::2] = -C.imag
    A[1::2, 0::2] = C.imag
    A[1::2, 1::2] = C.real
    return A


def _host_precompute(A0, params, omega, kappa, nonlinearity):
    T2_re, T2_im = _t2_like_reference(params, omega, kappa)
    L = T2_re.astype(np.float64) + 1j * T2_im.astype(np.float64)
    L = L + 1j * np.diag(omega.astype(np.float64))
    gh = float(nonlinearity[0]) * DT  # per-mode gamma*h (uniform here)

    Eh = _big_il(_expm(L * DT))
    Eh2 = _big_il(_expm(L * (DT / 2)))
    perm = np.arange(128) ^ 1
    P = np.eye(128)[perm]
    sgn = np.tile([1.0, -1.0], MODES)
    # per-mode gh (handles non-uniform nonlinearity too)
    ghv = np.repeat(nonlinearity.astype(np.float64) * DT, 2)

    def lt16(W):  # lhsT in fp16
        return np.ascontiguousarray(W.T, dtype=np.float16)

    # chain weights: bf16 hi/lo compensation (fp16 lo is ~96% subnormal and
    # gets flushed by the PE; bf16's exponent range avoids that entirely)
    import ml_dtypes
    bf16 = ml_dtypes.bfloat16
    EhT = np.ascontiguousarray(Eh.T, dtype=np.float32)
    wEhi = EhT.astype(bf16)
    wElo = (EhT - wEhi.astype(np.float32)).astype(bf16)

    # pair-sum + angle matrix: ss[p] = sgn[p]*gh[p]*(s2[2j]+s2[2j+1]), j=p//2
    Qg = np.zeros((128, 128), dtype=np.float64)
    for p in range(128):
        Qg[p, 2 * (p // 2)] = sgn[p] * ghv[p]
        Qg[p, 2 * (p // 2) + 1] = sgn[p] * ghv[p]

    # initial state, interleaved mode-major: (128, BATCH)
    y0 = np.zeros((2 * MODES, BATCH), dtype=np.float64)
    y0[0:2 * INPUT_MODES:2, :] = A0[:, :, 0].astype(np.float64).T
    y0[1:2 * INPUT_MODES:2, :] = A0[:, :, 1].astype(np.float64).T
    y0[2 * INPUT_MODES::2, :] = 1.0
    z0 = (Eh2 @ y0).astype(np.float32)
    y0M = y0.astype(np.float32)

    # cc for steps 0 and 1 (lag-2 bootstrap): from |z0|^2
    m2 = z0[0::2].astype(np.float64) ** 2 + z0[1::2].astype(np.float64) ** 2
    th = np.repeat(nonlinearity.astype(np.float64) * DT, 1)[:, None] * m2  # (64,B)
    cc0 = np.repeat(1.0 - 0.5 * th ** 2, 2, axis=0).astype(np.float32)

    # ACT Square scale for q = theta^2/2 from ss = +-theta: q = (scale*ss)^2
    qscale = float(1.0 / np.sqrt(2.0))

    return dict(wEhi=wEhi, wElo=wElo, wEhP=lt16(Eh @ P), wEh2=lt16(Eh2),
                wEh2P=lt16(Eh2 @ P), wQg=lt16(Qg),
                z0=z0, y0M=y0M, cc0=cc0, qscale=qscale)


# ---------------------------------------------------------------------------
# device kernel
# ---------------------------------------------------------------------------

def _build_nc(qscale):
    import concourse.bass as bass
    import concourse.bacc as bacc
    import concourse.tile as tile
    import concourse.mybir as mybir

    f32 = mybir.dt.float32
    f16 = mybir.dt.float16
    Square = mybir.ActivationFunctionType.Square
    Copy = mybir.ActivationFunctionType.Copy
    add = mybir.AluOpType.add
    mult = mybir.AluOpType.mult
    P = 128

    nc = bacc.Bacc("TRN2", target_bir_lowering=False, debug=False,
                   num_devices=N_CORES)

    bf16 = mybir.dt.bfloat16
    wn = ["wEhi", "wElo", "wEhP", "wEh2", "wEh2P", "wQg"]
    wdt = {"wEhi": bf16, "wElo": bf16}
    wd = {n: nc.dram_tensor(n, [P, P], wdt.get(n, f16), kind="ExternalInput").ap()
          for n in wn}
    z0_d = nc.dram_tensor("z0", [P, B_LOC], f32, kind="ExternalInput").ap()
    y0M_d = nc.dram_tensor("y0M", [P, B_LOC], f32, kind="ExternalInput").ap()
    cc0_d = nc.dram_tensor("cc0", [P, B_LOC], f32, kind="ExternalInput").ap()
    out_d = nc.dram_tensor("out", [EVAL_PTS, P, B_LOC], f32, kind="ExternalOutput").ap()

    with tile.TileContext(nc) as tc:
        with (
            tc.tile_pool(name="const", bufs=1) as cpool,
            tc.tile_pool(name="work", bufs=3) as npool,
            tc.tile_pool(name="ccp", bufs=3) as ccpool,
            tc.tile_pool(name="ocp", bufs=4) as opool,
            tc.tile_pool(name="pz", bufs=2, space="PSUM") as pzpool,
            tc.tile_pool(name="py", bufs=3, space="PSUM") as pypool,
            tc.tile_pool(name="pss", bufs=2, space="PSUM") as psspool,
            tc.tile_pool(name="pd", bufs=1, space="PSUM") as pdpool,
        ):
            wt = {}
            for n in wn:
                wt[n] = cpool.tile([P, P], wdt.get(n, f16), tag=n, name=n)
                nc.sync.dma_start(wt[n][:], wd[n][:])
            cc0_t = cpool.tile([P, B_LOC], f32, tag="cc0")
            nc.sync.dma_start(cc0_t[:], cc0_d[:])

            # t=0 output: pass-through of the initial state (mode-major)
            y0_t = cpool.tile([P, B_LOC], f32, tag="y0")
            nc.sync.dma_start(y0_t[:], y0M_d[:])
            nc.sync.dma_start(out_d[0], y0_t[:])

            # initial chain state z0 -> PSUM via a copy (so the loop body is
            # uniform: all steps read z from PSUM)
            z0_t = cpool.tile([P, B_LOC], f32, tag="z0")
            nc.sync.dma_start(z0_t[:], z0_d[:])
            z = pzpool.tile([P, B_LOC], f32, tag="z")
            nc.scalar.activation(z[:], z0_t[:], Copy)

            dscr = pdpool.tile([P, B_LOC], f32, tag="dscr")

            cc_tiles = [cc0_t, cc0_t]  # cc_0, cc_1
            q_tiles = [None] * NT

            for k in range(NT):
                last = (k == NT - 1)

                # cc_{k+1} (first V op of the step; inputs from step k-1,
                # ready long ago -> never stalls the V stream)
                if 1 <= k <= NT - 2:
                    ccn = ccpool.tile([P, B_LOC], f32, tag="cc")
                    nc.vector.tensor_scalar(ccn[:], q_tiles[k - 1][:], -1.0, 1.0,
                                            mult, add)
                    cc_tiles.append(ccn)
                cc_k = cc_tiles[k]

                # angle path: s2 -> ss (pair-sum matmul, gh*sgn in weights)
                s2 = npool.tile([P, B_LOC], f16, tag="s2")
                nc.scalar.activation(s2[:], z[:], Square)
                ss = psspool.tile([P, B_LOC], f32, tag="ss")
                nc.tensor.matmul(ss[:], wt["wQg"][:], s2[:], start=True, stop=True)
                # fp16 z copy: qt's z-factor (O(theta) correction term, fp16
                # rounding harmless); V may read only one PSUM operand, so qt
                # pairs z16(SBUF) with ss(PSUM)
                z16 = npool.tile([P, B_LOC], f16, tag="z16")
                nc.scalar.activation(z16[:], z[:], Copy)

                # products (V); pp first -- cc ready, z just landed
                pp = npool.tile([P, B_LOC], f16, tag="pp")
                nc.vector.tensor_tensor(pp[:], z[:], cc_k[:], mult)
                qt = npool.tile([P, B_LOC], f16, tag="qt")
                nc.vector.tensor_tensor(qt[:], z16[:], ss[:], mult)

                # chain update
                if not last:
                    zn = pzpool.tile([P, B_LOC], f32, tag="z")
                    nc.tensor.matmul(zn[:], wt["wEhi"][:], pp[:], start=True, stop=False)
                    nc.tensor.matmul(zn[:], wt["wElo"][:], pp[:], start=False, stop=False)
                    nc.tensor.matmul(zn[:], wt["wEhP"][:], qt[:], start=False, stop=True)

                # output y_{k+1}
                y = pypool.tile([P, B_LOC], f32, tag="y")
                nc.tensor.matmul(y[:], wt["wEh2"][:], pp[:], start=True, stop=False)
                nc.tensor.matmul(y[:], wt["wEh2P"][:], qt[:], start=False, stop=True)
                ysb = opool.tile([P, B_LOC], f32, tag="ysb")
                nc.scalar.activation(ysb[:], y[:], Copy)
                nc.sync.dma_start(out_d[k + 1], ysb[:])

                # lagged cc pipeline: q = (ss/sqrt2)^2 = theta^2/2
                if k <= NT - 3:
                    q = npool.tile([P, B_LOC], f32, tag="q")
                    nc.scalar.activation(q[:], ss[:], Square, 0.0, qscale)
                    q_tiles[k] = q

                # PE warm-up dummies: keep the HAM clock gate at 2.4 GHz
                # through the ACT/V phase of the next step
                if not last:
                    nc.tensor.matmul(dscr[:], wt["wQg"][:], pp[:], start=True, stop=True)
                    nc.tensor.matmul(dscr[:], wt["wQg"][:], qt[:], start=True, stop=True)
                    z = zn

    nc.compile()
    return nc


def _get_compiled(qscale=GH / np.sqrt(2.0)):
    if "nc" not in _CACHE:
        _CACHE["nc"] = _build_nc(float(qscale))
    return _CACHE["nc"]


def _run(host, trace=False, tmpdir=None):
    from concourse.bass_utils import run_bass_kernel_spmd

    nc = _get_compiled(host["qscale"])
    in_maps = []
    for i in range(N_CORES):
        sl = slice(i * B_LOC, (i + 1) * B_LOC)
        m = {n: host[n] for n in
             ["wEhi", "wElo", "wEhP", "wEh2", "wEh2P", "wQg"]}
        m["z0"] = np.ascontiguousarray(host["z0"][:, sl])
        m["y0M"] = np.ascontiguousarray(host["y0M"][:, sl])
        m["cc0"] = np.ascontiguousarray(host["cc0"][:, sl])
        in_maps.append(m)
    res = run_bass_kernel_spmd(nc, in_maps, list(range(N_CORES)), trace=trace,
                               tmpdir=tmpdir)
    full = np.empty((EVAL_PTS, BATCH, MODES, 2), dtype=np.float32)
    for i in range(N_CORES):
        sl = slice(i * B_LOC, (i + 1) * B_LOC)
        # core output is (t, 2j+c, b_local) -> (t, b_local, j, c)
        arr = res.results[i]["out"]
        full[:, sl, :, :] = arr.transpose(0, 2, 1).reshape(EVAL_PTS, B_LOC, MODES, 2)
    return full, res


def kernel(A0, params, omega, kappa, nonlinearity):
    A0 = np.asarray(A0, dtype=np.float32)
    params = np.asarray(params, dtype=np.float32)
    omega = np.asarray(omega, dtype=np.float32)
    kappa = np.asarray(kappa, dtype=np.float32)
    nonlinearity = np.asarray(nonlinearity, dtype=np.float32)

    host = _host_precompute(A0, params, omega, kappa, nonlinearity)
    full, _ = _run(host, trace=False)
    return full


# revision 11
# speedup vs baseline: 1.0025x; 1.0025x over previous
"""Trainium2 kernel for the nn_Circuit coupled-mode ODE problem.

Math: dA/dt = i*diag(omega + gamma*|A|^2) A + T2 A, integrated t in [0,2],
sampled at 200 points; A is (1024 batch, 64 modes) complex, padded with ones
for modes 48..63.  L = T2 + i*diag(omega) is constant, nearly skew-Hermitian.

Device algorithm: Strang splitting, linear part exact via host-precomputed
matrix exponentials, nonlinear part as a per-element phase rotation with
sin(theta) ~= theta (theta = gamma*h*|A|^2 <= ~1e-2, cubic error ~1e-7).
|A|^2 must come from the CURRENT state each step -- every lag/extrapolation/
predictor variant fails numerically (|A|^2 beats at ~2h period via the stiff
~288i eigenvalue; host-validated).  The per-step serial path is therefore

  z -> s2=Square(z) [ACT] -> ss=Qg@s2 [PE pair-sum matmul, gh*sgn folded
  into the weights] -> qt=z*ss [V] -> chain matmuls [PE]

with everything else off-path: pp = z*cc uses a 2-step-lagged cosine
cc = 1 - theta^2/2 (validated, the cos factor deviates from 1 by O(1e-4)),
computed via ACT Square of ss and a V affine; y output and its PSUM->SBUF
copy overlap the path.

    z_{k+1} = Eh_hi@pp + Eh_lo@pp + EhP@qt     (fp16, f32 PSUM accum)
    y_{k+1} = Eh2@pp + Eh2P@qt                 -> ACT copy -> DMA out

Chain weights Eh are hi/lo fp16 compensated; other weights single fp16
(operands O(theta) or output-only).  Host-validated rel err ~2.6e-3.

State layout: (128 partitions, 128 batch) f32 PSUM, partition p = 2j+c
interleaving re/im of mode j.  Output written mode-major per core and
transposed on the host during unsharding.

Sharding: pure data parallel, batch 1024 = 8 cores x 128.
"""

import numpy as np

MODES = 64
INPUT_MODES = 48
BATCH = 1024
EVAL_PTS = 200
EPS = 1e-8
N_CORES = 8
B_LOC = BATCH // N_CORES  # 128
NT = EVAL_PTS - 1  # 199 intervals
DT = 2.0 / NT
GAMMA = 0.2
GH = GAMMA * DT

_CACHE = {}

# ---------------------------------------------------------------------------
# host-side math
# ---------------------------------------------------------------------------

def _t2_like_reference(params, omega, kappa):
    """Reproduce the reference's float32 jax computation of T2 exactly."""
    import jax

    try:
        cpu = jax.devices("cpu")[0]
    except Exception:
        cpu = None

    import contextlib

    ctx = jax.default_device(cpu) if cpu is not None else contextlib.nullcontext()
    with ctx:
        import jax.numpy as jnp

        n = MODES
        p = jnp.asarray(params, dtype=jnp.float32)
        n_off = n * (n - 1) // 2
        iu = jnp.triu_indices(n, 1)
        off = p[:n_off] + 1j * p[n_off:2 * n_off]
        H = jnp.zeros((n, n), dtype=jnp.complex64).at[iu].set(off.astype(jnp.complex64))
        H = H + H.conj().T
        d = p[2 * n_off:]
        diag = jnp.concatenate([d, -jnp.sum(d, keepdims=True)])
        H = H + jnp.diag(diag.astype(jnp.complex64))
        U = jax.scipy.linalg.expm(1j * H)
        I = jnp.eye(n, dtype=jnp.complex64)
        M = U.T @ U
        mix = M @ jnp.linalg.inv(I - M + EPS * I)
        T2 = -jnp.asarray(kappa, dtype=jnp.float32) * (
            0.5 * jnp.eye(n, dtype=jnp.float32) + mix
        )
        T2_re = np.asarray(jnp.real(T2), dtype=np.float32)
        T2_im = np.asarray(jnp.imag(T2), dtype=np.float32)
    return T2_re, T2_im


def _expm(M):
    w, V = np.linalg.eig(M)
    return (V * np.exp(w)) @ np.linalg.inv(V)


def _big_il(C):
    """Complex (64,64) -> real (128,128) operator in the interleaved re/im basis."""
    A = np.zeros((2 * MODES, 2 * MODES), dtype=np.float64)
    A[0::2, 0::2] = C.real
    A[0::2, 1::2] = -C.imag
    A[1::2, 0::2] = C.imag
    A[1::2, 1::2] = C.real
    return A


def _host_precompute(A0, params, omega, kappa, nonlinearity):
    T2_re, T2_im = _t2_like_reference(params, omega, kappa)
    L = T2_re.astype(np.float64) + 1j * T2_im.astype(np.float64)
    L = L + 1j * np.diag(omega.astype(np.float64))
    gh = float(nonlinearity[0]) * DT  # per-mode gamma*h (uniform here)

    Eh = _big_il(_expm(L * DT))
    Eh2 = _big_il(_expm(L * (DT / 2)))
    perm = np.arange(128) ^ 1
    P = np.eye(128)[perm]
    sgn = np.tile([1.0, -1.0], MODES)
    # per-mode gh (handles non-uniform nonlinearity too)
    ghv = np.repeat(nonlinearity.astype(np.float64) * DT, 2)

    def lt16(W):  # lhsT in fp16
        return np.ascontiguousarray(W.T, dtype=np.float16)

    # chain weights: bf16 hi/lo compensation (fp16 lo is ~96% subnormal and
    # gets flushed by the PE; bf16's exponent range avoids that entirely)
    import ml_dtypes
    bf16 = ml_dtypes.bfloat16
    EhT = np.ascontiguousarray(Eh.T, dtype=np.float32)
    wEhi = EhT.astype(bf16)
    wElo = (EhT - wEhi.astype(np.float32)).astype(bf16)

    # pair-sum + angle matrix: ss[p] = sgn[p]*gh[p]*(s2[2j]+s2[2j+1]), j=p//2
    Qg = np.zeros((128, 128), dtype=np.float64)
    for p in range(128):
        Qg[p, 2 * (p // 2)] = sgn[p] * ghv[p]
        Qg[p, 2 * (p // 2) + 1] = sgn[p] * ghv[p]

    # initial state, interleaved mode-major: (128, BATCH)
    y0 = np.zeros((2 * MODES, BATCH), dtype=np.float64)
    y0[0:2 * INPUT_MODES:2, :] = A0[:, :, 0].astype(np.float64).T
    y0[1:2 * INPUT_MODES:2, :] = A0[:, :, 1].astype(np.float64).T
    y0[2 * INPUT_MODES::2, :] = 1.0
    z0 = (Eh2 @ y0).astype(np.float32)
    y0M = y0.astype(np.float32)

    # cc for steps 0 and 1 (lag-2 bootstrap): from |z0|^2
    m2 = z0[0::2].astype(np.float64) ** 2 + z0[1::2].astype(np.float64) ** 2
    th = np.repeat(nonlinearity.astype(np.float64) * DT, 1)[:, None] * m2  # (64,B)
    cc0 = np.repeat(1.0 - 0.5 * th ** 2, 2, axis=0).astype(np.float32)

    # ACT Square scale for q = theta^2/2 from ss = +-theta: q = (scale*ss)^2
    qscale = float(1.0 / np.sqrt(2.0))

    return dict(wEhi=wEhi, wElo=wElo, wEhP=lt16(Eh @ P), wEh2=lt16(Eh2),
                wEh2P=lt16(Eh2 @ P), wQg=lt16(Qg),
                z0=z0, y0M=y0M, cc0=cc0, qscale=qscale)


# ---------------------------------------------------------------------------
# device kernel
# ---------------------------------------------------------------------------

def _build_nc(qscale):
    import concourse.bass as bass
    import concourse.bacc as bacc
    import concourse.tile as tile
    import concourse.mybir as mybir

    f32 = mybir.dt.float32
    f16 = mybir.dt.float16
    Square = mybir.ActivationFunctionType.Square
    Copy = mybir.ActivationFunctionType.Copy
    add = mybir.AluOpType.add
    mult = mybir.AluOpType.mult
    P = 128

    nc = bacc.Bacc("TRN2", target_bir_lowering=False, debug=False,
                   num_devices=N_CORES)

    bf16 = mybir.dt.bfloat16
    wn = ["wEhi", "wElo", "wEhP", "wEh2", "wEh2P", "wQg"]
    wdt = {"wEhi": bf16, "wElo": bf16}
    wd = {n: nc.dram_tensor(n, [P, P], wdt.get(n, f16), kind="ExternalInput").ap()
          for n in wn}
    z0_d = nc.dram_tensor("z0", [P, B_LOC], f32, kind="ExternalInput").ap()
    y0M_d = nc.dram_tensor("y0M", [P, B_LOC], f32, kind="ExternalInput").ap()
    cc0_d = nc.dram_tensor("cc0", [P, B_LOC], f32, kind="ExternalInput").ap()
    out_d = nc.dram_tensor("out", [EVAL_PTS, P, B_LOC], f32, kind="ExternalOutput").ap()

    with tile.TileContext(nc) as tc:
        with (
            tc.tile_pool(name="const", bufs=1) as cpool,
            tc.tile_pool(name="work", bufs=3) as npool,
            tc.tile_pool(name="ccp", bufs=3) as ccpool,
            tc.tile_pool(name="ocp", bufs=4) as opool,
            tc.tile_pool(name="pz", bufs=2, space="PSUM") as pzpool,
            tc.tile_pool(name="py", bufs=3, space="PSUM") as pypool,
            tc.tile_pool(name="pss", bufs=2, space="PSUM") as psspool,
            tc.tile_pool(name="pd", bufs=1, space="PSUM") as pdpool,
        ):
            wt = {}
            for n in wn:
                wt[n] = cpool.tile([P, P], wdt.get(n, f16), tag=n, name=n)
                nc.sync.dma_start(wt[n][:], wd[n][:])
            cc0_t = cpool.tile([P, B_LOC], f32, tag="cc0")
            nc.sync.dma_start(cc0_t[:], cc0_d[:])

            # t=0 output: pass-through of the initial state (mode-major)
            y0_t = cpool.tile([P, B_LOC], f32, tag="y0")
            nc.sync.dma_start(y0_t[:], y0M_d[:])
            nc.sync.dma_start(out_d[0], y0_t[:])

            # initial chain state z0 -> PSUM via a copy (so the loop body is
            # uniform: all steps read z from PSUM)
            z0_t = cpool.tile([P, B_LOC], f32, tag="z0")
            nc.sync.dma_start(z0_t[:], z0_d[:])
            z = pzpool.tile([P, B_LOC], f32, tag="z")
            nc.scalar.activation(z[:], z0_t[:], Copy)

            dscr = pdpool.tile([P, B_LOC], f32, tag="dscr")

            cc_tiles = [cc0_t, cc0_t]  # cc_0, cc_1
            q_tiles = [None] * NT

            for k in range(NT):
                last = (k == NT - 1)

                # cc_{k+1} (first V op of the step; inputs from step k-1,
                # ready long ago -> never stalls the V stream)
                if 1 <= k <= NT - 2:
                    ccn = ccpool.tile([P, B_LOC], f32, tag="cc")
                    nc.vector.tensor_scalar(ccn[:], q_tiles[k - 1][:], -1.0, 1.0,
                                            mult, add)
                    cc_tiles.append(ccn)
                cc_k = cc_tiles[k]

                # angle path: s2 -> ss (pair-sum matmul, gh*sgn in weights)
                s2 = npool.tile([P, B_LOC], f16, tag="s2")
                nc.scalar.activation(s2[:], z[:], Square)
                ss = psspool.tile([P, B_LOC], f32, tag="ss")
                nc.tensor.matmul(ss[:], wt["wQg"][:], s2[:], start=True, stop=True)
                # fp16 z copy: qt's z-factor (O(theta) correction term, fp16
                # rounding harmless); V may read only one PSUM operand, so qt
                # pairs z16(SBUF) with ss(PSUM)
                z16 = npool.tile([P, B_LOC], f16, tag="z16")
                nc.scalar.activation(z16[:], z[:], Copy)

                # products (V); pp first -- cc ready, z just landed
                pp = npool.tile([P, B_LOC], f16, tag="pp")
                nc.vector.tensor_tensor(pp[:], z[:], cc_k[:], mult)
                qt = npool.tile([P, B_LOC], f16, tag="qt")
                nc.vector.tensor_tensor(qt[:], z16[:], ss[:], mult)

                # chain update
                if not last:
                    zn = pzpool.tile([P, B_LOC], f32, tag="z")
                    nc.tensor.matmul(zn[:], wt["wEhi"][:], pp[:], start=True, stop=False)
                    nc.tensor.matmul(zn[:], wt["wElo"][:], pp[:], start=False, stop=False)
                    nc.tensor.matmul(zn[:], wt["wEhP"][:], qt[:], start=False, stop=True)

                # output y_{k+1}
                y = pypool.tile([P, B_LOC], f32, tag="y")
                nc.tensor.matmul(y[:], wt["wEh2"][:], pp[:], start=True, stop=False)
                nc.tensor.matmul(y[:], wt["wEh2P"][:], qt[:], start=False, stop=True)
                ysb = opool.tile([P, B_LOC], f32, tag="ysb")
                nc.scalar.activation(ysb[:], y[:], Copy)
                nc.sync.dma_start(out_d[k + 1], ysb[:])

                # lagged cc pipeline: q = (ss/sqrt2)^2 = theta^2/2
                if k <= NT - 3:
                    q = npool.tile([P, B_LOC], f32, tag="q")
                    nc.scalar.activation(q[:], ss[:], Square, 0.0, qscale)
                    q_tiles[k] = q

                # PE warm-up dummies: keep the HAM clock gate at 2.4 GHz
                # through the ACT/V phase of the next step
                if not last:
                    nc.tensor.matmul(dscr[:], wt["wQg"][:], pp[:], start=True, stop=True)
                    nc.tensor.matmul(dscr[:], wt["wQg"][:], qt[:], start=True, stop=True)
                    z = zn

    nc.compile()
    return nc


def _get_compiled(qscale=1.0 / np.sqrt(2.0)):
    if "nc" not in _CACHE:
        _CACHE["nc"] = _build_nc(float(qscale))
    return _CACHE["nc"]


def _run(host, trace=False, tmpdir=None):
    from concourse.bass_utils import run_bass_kernel_spmd

    nc = _get_compiled(host["qscale"])
    in_maps = []
    for i in range(N_CORES):
        sl = slice(i * B_LOC, (i + 1) * B_LOC)
        m = {n: host[n] for n in
             ["wEhi", "wElo", "wEhP", "wEh2", "wEh2P", "wQg"]}
        m["z0"] = np.ascontiguousarray(host["z0"][:, sl])
        m["y0M"] = np.ascontiguousarray(host["y0M"][:, sl])
        m["cc0"] = np.ascontiguousarray(host["cc0"][:, sl])
        in_maps.append(m)
    res = run_bass_kernel_spmd(nc, in_maps, list(range(N_CORES)), trace=trace,
                               tmpdir=tmpdir)
    full = np.empty((EVAL_PTS, BATCH, MODES, 2), dtype=np.float32)
    for i in range(N_CORES):
        sl = slice(i * B_LOC, (i + 1) * B_LOC)
        # core output is (t, 2j+c, b_local) -> (t, b_local, j, c)
        arr = res.results[i]["out"]
        full[:, sl, :, :] = arr.transpose(0, 2, 1).reshape(EVAL_PTS, B_LOC, MODES, 2)
    return full, res


def kernel(A0, params, omega, kappa, nonlinearity):
    A0 = np.asarray(A0, dtype=np.float32)
    params = np.asarray(params, dtype=np.float32)
    omega = np.asarray(omega, dtype=np.float32)
    kappa = np.asarray(kappa, dtype=np.float32)
    nonlinearity = np.asarray(nonlinearity, dtype=np.float32)

    host = _host_precompute(A0, params, omega, kappa, nonlinearity)
    full, _ = _run(host, trace=False)
    return full


# revision 13
# speedup vs baseline: 1.4480x; 1.4444x over previous
"""Trainium2 kernel for the nn_Circuit coupled-mode ODE problem.

Math: dA/dt = i*diag(omega + gamma*|A|^2) A + T2 A, integrated t in [0,2],
sampled at 200 points; A is (1024 batch, 64 modes) complex, padded with ones
for modes 48..63.  L = T2 + i*diag(omega) is constant, nearly skew-Hermitian.

Device algorithm: Strang splitting, linear part exact via host-precomputed
matrix exponentials, nonlinear part as a per-element phase rotation with
sin(theta) ~= theta (theta = gamma*h*|A|^2 <= ~1e-2, cubic error ~1e-7).
|A|^2 must come from the CURRENT state each step -- every lag/extrapolation/
predictor variant fails numerically (|A|^2 beats at ~2h period via the stiff
~288i eigenvalue; host-validated).  The per-step serial path is therefore

  z -> s2=Square(z) [ACT] -> ss=Qg@s2 [PE pair-sum matmul, gh*sgn folded
  into the weights] -> qt=z*ss [V] -> chain matmuls [PE]

with everything else off-path: pp = z*cc uses a 2-step-lagged cosine
cc = 1 - theta^2/2 (validated, the cos factor deviates from 1 by O(1e-4)),
computed via ACT Square of ss and a V affine; y output and its PSUM->SBUF
copy overlap the path.

    z_{k+1} = Eh_hi@pp + Eh_lo@pp + EhP@qt     (fp16, f32 PSUM accum)
    y_{k+1} = Eh2@pp + Eh2P@qt                 -> ACT copy -> DMA out

Chain weights Eh are hi/lo fp16 compensated; other weights single fp16
(operands O(theta) or output-only).  Host-validated rel err ~2.6e-3.

State layout: (128 partitions, 128 batch) f32 PSUM, partition p = 2j+c
interleaving re/im of mode j.  Output written mode-major per core and
transposed on the host during unsharding.

Sharding: pure data parallel, batch 1024 = 8 cores x 128.
"""

import numpy as np

MODES = 64
INPUT_MODES = 48
BATCH = 1024
EVAL_PTS = 200
EPS = 1e-8
N_CORES = 8
B_LOC = BATCH // N_CORES  # 128
NT = EVAL_PTS - 1  # 199 intervals
DT = 2.0 / NT
GAMMA = 0.2
GH = GAMMA * DT

_CACHE = {}

# ---------------------------------------------------------------------------
# host-side math
# ---------------------------------------------------------------------------

def _t2_like_reference(params, omega, kappa):
    """Reproduce the reference's float32 jax computation of T2 exactly."""
    import jax

    try:
        cpu = jax.devices("cpu")[0]
    except Exception:
        cpu = None

    import contextlib

    ctx = jax.default_device(cpu) if cpu is not None else contextlib.nullcontext()
    with ctx:
        import jax.numpy as jnp

        n = MODES
        p = jnp.asarray(params, dtype=jnp.float32)
        n_off = n * (n - 1) // 2
        iu = jnp.triu_indices(n, 1)
        off = p[:n_off] + 1j * p[n_off:2 * n_off]
        H = jnp.zeros((n, n), dtype=jnp.complex64).at[iu].set(off.astype(jnp.complex64))
        H = H + H.conj().T
        d = p[2 * n_off:]
        diag = jnp.concatenate([d, -jnp.sum(d, keepdims=True)])
        H = H + jnp.diag(diag.astype(jnp.complex64))
        U = jax.scipy.linalg.expm(1j * H)
        I = jnp.eye(n, dtype=jnp.complex64)
        M = U.T @ U
        mix = M @ jnp.linalg.inv(I - M + EPS * I)
        T2 = -jnp.asarray(kappa, dtype=jnp.float32) * (
            0.5 * jnp.eye(n, dtype=jnp.float32) + mix
        )
        T2_re = np.asarray(jnp.real(T2), dtype=np.float32)
        T2_im = np.asarray(jnp.imag(T2), dtype=np.float32)
    return T2_re, T2_im


def _expm(M):
    w, V = np.linalg.eig(M)
    return (V * np.exp(w)) @ np.linalg.inv(V)


def _big_il(C):
    """Complex (64,64) -> real (128,128) operator in the interleaved re/im basis."""
    A = np.zeros((2 * MODES, 2 * MODES), dtype=np.float64)
    A[0::2, 0::2] = C.real
    A[0::2, 1::2] = -C.imag
    A[1::2, 0::2] = C.imag
    A[1::2, 1::2] = C.real
    return A


def _host_precompute(A0, params, omega, kappa, nonlinearity):
    T2_re, T2_im = _t2_like_reference(params, omega, kappa)
    L = T2_re.astype(np.float64) + 1j * T2_im.astype(np.float64)
    L = L + 1j * np.diag(omega.astype(np.float64))
    gh = float(nonlinearity[0]) * DT  # per-mode gamma*h (uniform here)

    Eh = _big_il(_expm(L * DT))
    Eh2 = _big_il(_expm(L * (DT / 2)))
    perm = np.arange(128) ^ 1
    P = np.eye(128)[perm]
    sgn = np.tile([1.0, -1.0], MODES)
    # per-mode gh (handles non-uniform nonlinearity too)
    ghv = np.repeat(nonlinearity.astype(np.float64) * DT, 2)

    def lt16(W):  # lhsT in fp16
        return np.ascontiguousarray(W.T, dtype=np.float16)

    # chain weights: bf16 hi/lo compensation (fp16 lo is ~96% subnormal and
    # gets flushed by the PE; bf16's exponent range avoids that entirely)
    import ml_dtypes
    bf16 = ml_dtypes.bfloat16
    EhT = np.ascontiguousarray(Eh.T, dtype=np.float32)
    wEhi = EhT.astype(bf16)
    wElo = (EhT - wEhi.astype(np.float32)).astype(bf16)

    # pair-sum + angle matrix: ss[p] = sgn[p]*gh[p]*(s2[2j]+s2[2j+1]), j=p//2
    Qg = np.zeros((128, 128), dtype=np.float64)
    for p in range(128):
        Qg[p, 2 * (p // 2)] = sgn[p] * ghv[p]
        Qg[p, 2 * (p // 2) + 1] = sgn[p] * ghv[p]

    # initial state, interleaved mode-major: (128, BATCH)
    y0 = np.zeros((2 * MODES, BATCH), dtype=np.float64)
    y0[0:2 * INPUT_MODES:2, :] = A0[:, :, 0].astype(np.float64).T
    y0[1:2 * INPUT_MODES:2, :] = A0[:, :, 1].astype(np.float64).T
    y0[2 * INPUT_MODES::2, :] = 1.0
    z0 = (Eh2 @ y0).astype(np.float32)
    y0M = y0.astype(np.float32)

    # cc for steps 0 and 1 (lag-2 bootstrap): from |z0|^2
    m2 = z0[0::2].astype(np.float64) ** 2 + z0[1::2].astype(np.float64) ** 2
    th = np.repeat(nonlinearity.astype(np.float64) * DT, 1)[:, None] * m2  # (64,B)
    cc0 = np.repeat(1.0 - 0.5 * th ** 2, 2, axis=0).astype(np.float32)

    # ACT Square scale for q = theta^2/2 from ss = +-theta: q = (scale*ss)^2
    qscale = float(1.0 / np.sqrt(2.0))

    return dict(wEhi=wEhi, wElo=wElo, wEhP=lt16(Eh @ P), wEh2=lt16(Eh2),
                wEh2P=lt16(Eh2 @ P), wQg=lt16(Qg),
                z0=z0, y0M=y0M, cc0=cc0, qscale=qscale)


# ---------------------------------------------------------------------------
# device kernel
# ---------------------------------------------------------------------------

def _build_nc(qscale):
    import concourse.bass as bass
    import concourse.bacc as bacc
    import concourse.tile as tile
    import concourse.mybir as mybir

    f32 = mybir.dt.float32
    f16 = mybir.dt.float16
    Square = mybir.ActivationFunctionType.Square
    Copy = mybir.ActivationFunctionType.Copy
    add = mybir.AluOpType.add
    mult = mybir.AluOpType.mult
    P = 128

    nc = bacc.Bacc("TRN2", target_bir_lowering=False, debug=False,
                   num_devices=N_CORES)

    bf16 = mybir.dt.bfloat16
    wn = ["wEhi", "wElo", "wEhP", "wEh2", "wEh2P", "wQg"]
    wdt = {"wEhi": bf16, "wElo": bf16}
    wd = {n: nc.dram_tensor(n, [P, P], wdt.get(n, f16), kind="ExternalInput").ap()
          for n in wn}
    z0_d = nc.dram_tensor("z0", [P, B_LOC], f32, kind="ExternalInput").ap()
    y0M_d = nc.dram_tensor("y0M", [P, B_LOC], f32, kind="ExternalInput").ap()
    cc0_d = nc.dram_tensor("cc0", [P, B_LOC], f32, kind="ExternalInput").ap()
    out_d = nc.dram_tensor("out", [EVAL_PTS, P, B_LOC], f32, kind="ExternalOutput").ap()

    with tile.TileContext(nc) as tc:
        with (
            tc.tile_pool(name="const", bufs=1) as cpool,
            tc.tile_pool(name="work", bufs=3) as npool,
            tc.tile_pool(name="ccp", bufs=3) as ccpool,
            tc.tile_pool(name="ocp", bufs=4) as opool,
            tc.tile_pool(name="pz", bufs=2, space="PSUM") as pzpool,
            tc.tile_pool(name="py", bufs=3, space="PSUM") as pypool,
            tc.tile_pool(name="pss", bufs=2, space="PSUM") as psspool,
            tc.tile_pool(name="pd", bufs=1, space="PSUM") as pdpool,
        ):
            wt = {}
            for n in wn:
                wt[n] = cpool.tile([P, P], wdt.get(n, f16), tag=n, name=n)
                nc.sync.dma_start(wt[n][:], wd[n][:])
            cc0_t = cpool.tile([P, B_LOC], f32, tag="cc0")
            nc.sync.dma_start(cc0_t[:], cc0_d[:])

            # t=0 output: pass-through of the initial state (mode-major)
            y0_t = cpool.tile([P, B_LOC], f32, tag="y0")
            nc.sync.dma_start(y0_t[:], y0M_d[:])
            nc.sync.dma_start(out_d[0], y0_t[:])

            # initial chain state z0 -> PSUM via a copy (so the loop body is
            # uniform: all steps read z from PSUM)
            z0_t = cpool.tile([P, B_LOC], f32, tag="z0")
            nc.sync.dma_start(z0_t[:], z0_d[:])
            z = pzpool.tile([P, B_LOC], f32, tag="z")
            nc.scalar.activation(z[:], z0_t[:], Copy)

            dscr = pdpool.tile([P, B_LOC], f32, tag="dscr")

            cc_tiles = [cc0_t, cc0_t]  # cc_0, cc_1
            q_tiles = [None] * NT

            y_pend = None  # deferred (y_psum, time_index) for the ACT copy
            pp_prev = wt["wEh2"]  # any ready fp16 SBUF tile as dummy-matmul rhs

            for k in range(NT):
                last = (k == NT - 1)

                # cc_{k+1} (first V op of the step; inputs from step k-1,
                # ready long ago -> never stalls the V stream)
                if 1 <= k <= NT - 2:
                    ccn = ccpool.tile([P, B_LOC], f32, tag="cc")
                    nc.vector.tensor_scalar(ccn[:], q_tiles[k - 1][:], -1.0, 1.0,
                                            mult, add)
                    cc_tiles.append(ccn)
                cc_k = cc_tiles[k]

                # PE warm-up dummies fill the s2-wait gap so the path matmuls
                # run at a warm HAM pstate (narrow rhs: LD dominates anyway)
                for _ in range(3):
                    nc.tensor.matmul(dscr[:, :32], wt["wQg"][:], pp_prev[:, :32],
                                     start=True, stop=True)

                # angle path: s2 -> ss (pair-sum matmul, gh*sgn in weights)
                s2 = npool.tile([P, B_LOC], f16, tag="s2")
                nc.scalar.activation(s2[:], z[:], Square)
                ss = psspool.tile([P, B_LOC], f32, tag="ss")
                nc.tensor.matmul(ss[:], wt["wQg"][:], s2[:], start=True, stop=True)

                # products (V); pp first -- cc ready, z just landed.  qt uses
                # pp as its z-factor (qt = z*cc*ss, |cc-1|~1e-5 on an O(theta)
                # term): keeps qt's PSUM operand count at 1 with no ACT copy.
                pp = npool.tile([P, B_LOC], f16, tag="pp")
                nc.vector.tensor_tensor(pp[:], z[:], cc_k[:], mult)
                qt = npool.tile([P, B_LOC], f16, tag="qt")
                nc.vector.tensor_tensor(qt[:], pp[:], ss[:], mult)

                # chain update
                if not last:
                    zn = pzpool.tile([P, B_LOC], f32, tag="z")
                    nc.tensor.matmul(zn[:], wt["wEhi"][:], pp[:], start=True, stop=False)
                    nc.tensor.matmul(zn[:], wt["wElo"][:], pp[:], start=False, stop=False)
                    nc.tensor.matmul(zn[:], wt["wEhP"][:], qt[:], start=False, stop=True)

                # output y_{k+1}
                y = pypool.tile([P, B_LOC], f32, tag="y")
                nc.tensor.matmul(y[:], wt["wEh2"][:], pp[:], start=True, stop=False)
                nc.tensor.matmul(y[:], wt["wEh2P"][:], qt[:], start=False, stop=True)

                # lagged cc pipeline: q = (ss/sqrt2)^2 = theta^2/2
                if k <= NT - 3:
                    q = npool.tile([P, B_LOC], f32, tag="q")
                    nc.scalar.activation(q[:], ss[:], Square, 0.0, qscale)
                    q_tiles[k] = q

                # previous step's output copy + DMA, deferred so the in-order
                # ACT queue never delays the critical s2 of the next step
                if y_pend is not None:
                    yp, ti = y_pend
                    ysb = opool.tile([P, B_LOC], f32, tag="ysb")
                    nc.scalar.activation(ysb[:], yp[:], Copy)
                    nc.sync.dma_start(out_d[ti], ysb[:])
                y_pend = (y, k + 1)

                if not last:
                    z = zn
                pp_prev = pp

            yp, ti = y_pend
            ysb = opool.tile([P, B_LOC], f32, tag="ysb")
            nc.scalar.activation(ysb[:], yp[:], Copy)
            nc.sync.dma_start(out_d[ti], ysb[:])

    nc.compile()
    return nc


def _get_compiled(qscale=1.0 / np.sqrt(2.0)):
    if "nc" not in _CACHE:
        _CACHE["nc"] = _build_nc(float(qscale))
    return _CACHE["nc"]


def _run(host, trace=False, tmpdir=None):
    from concourse.bass_utils import run_bass_kernel_spmd

    nc = _get_compiled(host["qscale"])
    in_maps = []
    for i in range(N_CORES):
        sl = slice(i * B_LOC, (i + 1) * B_LOC)
        m = {n: host[n] for n in
             ["wEhi", "wElo", "wEhP", "wEh2", "wEh2P", "wQg"]}
        m["z0"] = np.ascontiguousarray(host["z0"][:, sl])
        m["y0M"] = np.ascontiguousarray(host["y0M"][:, sl])
        m["cc0"] = np.ascontiguousarray(host["cc0"][:, sl])
        in_maps.append(m)
    res = run_bass_kernel_spmd(nc, in_maps, list(range(N_CORES)), trace=trace,
                               tmpdir=tmpdir)
    full = np.empty((EVAL_PTS, BATCH, MODES, 2), dtype=np.float32)
    for i in range(N_CORES):
        sl = slice(i * B_LOC, (i + 1) * B_LOC)
        # core output is (t, 2j+c, b_local) -> (t, b_local, j, c)
        arr = res.results[i]["out"]
        full[:, sl, :, :] = arr.transpose(0, 2, 1).reshape(EVAL_PTS, B_LOC, MODES, 2)
    return full, res


def kernel(A0, params, omega, kappa, nonlinearity):
    A0 = np.asarray(A0, dtype=np.float32)
    params = np.asarray(params, dtype=np.float32)
    omega = np.asarray(omega, dtype=np.float32)
    kappa = np.asarray(kappa, dtype=np.float32)
    nonlinearity = np.asarray(nonlinearity, dtype=np.float32)

    host = _host_precompute(A0, params, omega, kappa, nonlinearity)
    full, _ = _run(host, trace=False)
    return full
